# revision 8
# baseline (speedup 1.0000x reference)
"""MoE-with-DeepGEMM kernel for 8 Trainium2 NeuronCores.

Problem: M=4096 tokens, D=2048 in-dim, H=2048 out-dim, E=8 experts.
    gate = softmax(x @ gate_w.T + gate_b)            # [M, E], fp32
    y    = (q8(x) @ q8(expert_w[e]).T) -> bf16       # [E, M, H]
    out  = sum_e gate[:, e, None] * y[e].astype(f32) # [M, H]

Strategy: data-parallel over tokens (M). Each of the 8 cores gets
M/8 = 512 tokens, all 8 experts' weights, and computes its output slice
independently — no collectives; the host concatenates the slices.

The PE issue rate is the wall: 1024 DoubleRow matmuls x 216 ns is
~221 us of tensor-engine time; the schedule's job is to keep the PE
issuing back-to-back from as early as possible and to keep everything
else off its critical path:
  - 18 dummy warm-up matmuls on a memset scratch tile run right after
    the framework preamble so HAM un-throttles the PE (K=8/8) before
    the real stream starts.
  - DMA launches spread over three launch queues (Sync/GpSimd/Scalar,
    ~0.6-1 us serialized launch cost each). The first-needed pieces
    (xq head k-pair, w0's first h-half k-pieces) launch first and are
    NOT contended: bulk pieces (w0's second h-half, xf, w1) are gated
    on early-piece completion semaphores so their packets don't steal
    HBM bandwidth from the critical ladder.
  - e0 runs k-major in two h-half phases (all 4 mc x 2 hc = 8 PSUM
    banks per phase), which halves its weight-consumption rate to
    ~148 GB/s — below the early-window HBM supply — so the PE never
    stalls on w0 arrival. PSUM tiles are ACT-copied UNSCALED into the
    f32 accumulator (gate isn't known yet); after gating+softmax, acc
    is scaled in place by gate[:,0] (DVE), interleaved into e1.
  - Gating matmuls (fp16, N=512) at the e0/e1 boundary; the PE softmax
    transposes are deferred past e1-mc0's matmuls so the PE never
    waits on the DVE logits copy.
  - e1..e6: weighted combine acc += gate_e * psum as one DVE
    scalar_tensor_tensor straight from PSUM (no intermediate bf16
    copy). PSUM banks recycle two mc-windows later — ~7 us of slack.
  - e7: per (mc,hc) tile, stt writes gate_7*psum + acc into a bf16
    tile DMA'd out immediately; the 16 output launches alternate over
    the three launch queues so the final tile's launch isn't queued
    behind the others. Kernel tail = last stt + one 128KB DMA.

Host-side prep (not device work): fp8 quantize (identical RNE cast the
reference performs), transposes so the contraction dim lands on SBUF
partitions, bf16->f32 upcast of the output and the final concat.
"""

import numpy as np
import ml_dtypes

import concourse.bacc as bacc
import concourse.bass as bass
import concourse.mybir as mybir
import concourse.tile as tile
from concourse import masks
from concourse.tile import add_dep_helper
from concourse.bass_utils import run_bass_kernel_spmd

M, D, H, E = 4096, 2048, 2048, 8
NCORES = 8
MS = M // NCORES          # tokens per core (512)
MC = MS // 128            # m-chunks of 128 partitions (4)
DS = D // 128             # d-subtiles of 128 (16)
KP = DS // 2              # DoubleRow d-pairs of 256 (8)
NH = 512                  # h columns per matmul (one PSUM bank of f32)
HC = H // NH              # h-chunks (4)
N_WARM = 18               # dummy warm-up matmuls (N=128) for HAM ramp

_NC = None


def _build_program() -> bass.Bass:
    dt = mybir.dt
    nc = bacc.Bacc(None, target_bir_lowering=False)

    xq = nc.dram_tensor("xq", [D, MS], dt.float8e4, kind="ExternalInput")
    xf = nc.dram_tensor("xf", [D, MS], dt.float16, kind="ExternalInput")
    wq = nc.dram_tensor("wq", [E * D, H], dt.float8e4, kind="ExternalInput")
    gwt = nc.dram_tensor("gwt", [D, E], dt.float16, kind="ExternalInput")
    gb = nc.dram_tensor("gb", [E, 1], dt.float32, kind="ExternalInput")
    out = nc.dram_tensor("out", [MS, H], dt.bfloat16, kind="ExternalOutput")

    with tile.TileContext(nc) as tc, \
            tc.tile_pool(name="const", bufs=1) as constp, \
            tc.tile_pool(name="wpool", bufs=2) as wpool, \
            tc.tile_pool(name="outp", bufs=4) as outp, \
            tc.tile_pool(name="small", bufs=8) as small, \
            tc.tile_pool(name="ps", bufs=8, space="PSUM") as psp:

        # Persistent SBUF tensors. Contraction index d = s*128 + p.
        xq_sb = constp.tile([128, DS, MS], dt.float8e4, tag="xq")
        xf_sb = constp.tile([128, DS, MS], dt.float16, tag="xf")
        gwt_sb = constp.tile([128, DS, E], dt.float16, tag="gwt")
        gb_sb = constp.tile([E, 1], dt.float32, tag="gb")
        id8_sb = constp.tile([E, E], dt.float32, tag="id8")
        gate_sb = constp.tile([128, MC * E], dt.float32, tag="gate")
        lg_sb = constp.tile([E, MS], dt.float32, tag="lg")
        acc_sb = constp.tile([128, MC * H], dt.float32, tag="acc")
        warm_sb = constp.tile([128, 256], dt.bfloat16, tag="warm")

        masks.make_identity(nc, id8_sb[:])
        nc.gpsimd.memset(warm_sb[:], 0.25)

        # PE warm-up: keep the tensor engine busy from t~7.5us (end of
        # the framework preamble) so HAM reaches K=8/8 before the real
        # matmul stream begins.
        ps_warm = psp.tile([128, 128], dt.float32, tag="ps", name="ps_warm")
        for _ in range(N_WARM):
            nc.tensor.matmul(
                ps_warm[:], lhsT=warm_sb[:, 0:128], rhs=warm_sb[:, 128:256],
                start=True, stop=True,
            )

        def rr(src):
            return src.rearrange("(s p) m -> p s m", p=128)

        def rw(src):
            return src.rearrange("(s p) h -> p s h", p=128)

        # ---- DMA ladder ----
        # Sync launch queue (fastest starter): the exact bytes the first
        # matmuls need, smallest pieces first, then w0's first h-half
        # k-major. The first DR matmul needs only xq[s0:2] + w0[s0:2,
        # h0:512] = 256KB.
        w_sb0 = wpool.tile([128, DS, H], dt.float8e4, tag="w")
        nc.sync.dma_start(xq_sb[:, 0:2, :], rr(xq[0:256, :]))
        nc.sync.dma_start(w_sb0[:, 0:2, 0:512], wq[0:256, 0:512].rearrange(
            "(s p) h -> p s h", p=128))
        nc.sync.dma_start(w_sb0[:, 0:2, 512:1024], wq[0:256, 512:1024].rearrange(
            "(s p) h -> p s h", p=128))
        d_w0a = []
        for r0, r1 in ((256, 512), (512, 1024), (1024, 1536), (1536, 2048)):
            dj = nc.sync.dma_start(
                w_sb0[:, r0 // 128:r1 // 128, 0:1024], rw(wq[r0:r1, 0:1024])
            )
            d_w0a.append(dj)
        # GpSimd launch queue: xq tail, gb, gwt.
        nc.gpsimd.dma_start(xq_sb[:, 2:8, :], rr(xq[256:1024, :]))
        nc.gpsimd.dma_start(xq_sb[:, 8:DS, :], rr(xq[1024:D, :]))
        nc.gpsimd.dma_start(gb_sb[:], gb[:, :])
        nc.gpsimd.dma_start(gwt_sb[:], gwt[:, :].rearrange("(s p) e -> p s e", p=128))
        # Scalar launch queue: w0's second h-half, gated on first-half
        # progress so its packets don't starve the critical ladder.
        d_w0b = []
        for k, (r0, r1) in enumerate(((0, 512), (512, 1024),
                                      (1024, 1536), (1536, 2048))):
            dj = nc.scalar.dma_start(
                w_sb0[:, r0 // 128:r1 // 128, 1024:2048], rw(wq[r0:r1, 1024:2048])
            )
            add_dep_helper(dj.ins, d_w0a[min(k + 1, 3)].ins,
                           reason="w0b after w0a ladder")
            d_w0b.append(dj)
        # xf (gating input): gated behind w0b; needed only at the e0/e1
        # boundary (~40us in).
        d_xfs = []
        for j in range(2):
            dj = nc.gpsimd.dma_start(
                xf_sb[:, j * 8:(j + 1) * 8, :],
                rr(xf[j * 1024:(j + 1) * 1024, :]),
            )
            add_dep_helper(dj.ins, d_w0b[1 + j].ins, reason="xf after w0b")
            d_xfs.append(dj)

        # ---- Expert 0: k-major in two h-half phases (all mc x 2 hc =
        # 8 PSUM banks each). Halves the w0 consumption rate so the PE
        # never outruns the early DMA supply. PSUM -> acc_sb UNSCALED
        # via ACT copies; gate[:,0] scale applied in place later.
        for phase, hcs in enumerate(((0, 1), (2, 3))):
            pss = {
                mc: {
                    hc: psp.tile([128, NH], dt.float32, tag="ps",
                                 name=f"ps0_{mc}_{hc}")
                    for hc in hcs
                }
                for mc in range(MC)
            }
            for k in range(KP):
                # First k-step of the kernel consumes its two 128KB w
                # pieces in arrival order: all mc for hc0, then hc1.
                if phase == 0 and k == 0:
                    mc_hc = [(mc, hc) for hc in hcs for mc in range(MC)]
                else:
                    mc_hc = [(mc, hc) for mc in range(MC) for hc in hcs]
                for mc, hc in mc_hc:
                    lhsT = xq_sb[:, 2 * k:2 * k + 2, mc * 128:(mc + 1) * 128]
                    nc.tensor.matmul(
                        pss[mc][hc][:],
                        lhsT=lhsT,
                        rhs=w_sb0[:, 2 * k:2 * k + 2, hc * NH:(hc + 1) * NH],
                        start=(k == 0),
                        stop=(k == KP - 1),
                        perf_mode=mybir.MatmulPerfMode.DoubleRow,
                    )
            for mc in range(MC):
                for hc in hcs:
                    nc.scalar.copy(
                        acc_sb[:, mc * H + hc * NH:mc * H + (hc + 1) * NH],
                        pss[mc][hc][:],
                    )

        # ---- Gating matmuls at the e0/e1 boundary ----
        ps_gt = psp.tile([E, MS], dt.float32, tag="ps", name="ps_gt")
        for s in range(DS):
            nc.tensor.matmul(
                ps_gt[:],
                lhsT=gwt_sb[:, s:s + 1, :],
                rhs=xf_sb[:, s:s + 1, :],
                start=(s == 0),
                stop=(s == DS - 1),
            )
        nc.vector.tensor_scalar_add(lg_sb[:], ps_gt[:], gb_sb[:])

        def emit_softmax():
            for mc in range(MC):
                pst = psp.tile([128, E], dt.float32, tag="ps", name=f"ps_t{mc}")
                nc.tensor.transpose(
                    pst[:], lg_sb[:, mc * 128:(mc + 1) * 128], id8_sb[:]
                )
                mx = small.tile([128, 1], dt.float32, tag="sm1")
                nc.vector.tensor_reduce(
                    mx[:], pst[:], mybir.AxisListType.X, mybir.AluOpType.max
                )
                nmx = small.tile([128, 1], dt.float32, tag="sm1")
                nc.vector.tensor_scalar_mul(nmx[:], mx[:], -1.0)
                ex = small.tile([128, E], dt.float32, tag="sm")
                ssum = small.tile([128, 1], dt.float32, tag="sm1")
                nc.scalar.activation(
                    ex[:], pst[:], mybir.ActivationFunctionType.Exp,
                    bias=nmx[:], scale=1.0, accum_out=ssum[:],
                )
                rcp = small.tile([128, 1], dt.float32, tag="sm1")
                nc.vector.reciprocal(rcp[:], ssum[:])
                nc.vector.tensor_scalar_mul(
                    gate_sb[:, mc * E:(mc + 1) * E], ex[:], rcp[:]
                )

        # ---- Experts 1..7: mc-major, DVE combine straight from PSUM ----
        # Output-launch queues: GpSimd only gets early tiles (its
        # end-of-kernel queue drain would otherwise serialize the
        # teardown behind a late transfer); the last mc alternates
        # Scalar/Sync so the final tile's launch waits only on its own
        # stt.
        out_q = {
            0: [nc.gpsimd, nc.gpsimd, nc.gpsimd, nc.gpsimd],
            1: [nc.scalar, nc.scalar, nc.scalar, nc.scalar],
            2: [nc.sync, nc.sync, nc.gpsimd, nc.scalar],
            3: [nc.scalar, nc.sync, nc.scalar, nc.sync],
        }
        d_w_first = None
        for e in range(1, E):
            w_sb = wpool.tile([128, DS, H], dt.float8e4, tag="w")
            for j in range(2):
                rsl = slice(e * D + j * (D // 2), e * D + (j + 1) * (D // 2))
                dw = nc.sync.dma_start(
                    w_sb[:, j * (DS // 2):(j + 1) * (DS // 2), :],
                    wq[rsl, :].rearrange("(s p) h -> p s h", p=128),
                )
                if e == 1 and j == 0:
                    # Keep w1's packets out of the pool until most of
                    # the w0/xq front-load has drained.
                    add_dep_helper(dw.ins, d_w0b[2].ins, reason="w1 after w0b")
            for mc in range(MC):
                msl = slice(mc * 128, (mc + 1) * 128)
                pss = [
                    psp.tile([128, NH], dt.float32, tag="ps", name=f"ps_{e}_{mc}_{i}")
                    for i in range(HC)
                ]
                for k in range(KP):
                    lhsT = xq_sb[:, 2 * k:2 * k + 2, msl]
                    for hc in range(HC):
                        nc.tensor.matmul(
                            pss[hc][:],
                            lhsT=lhsT,
                            rhs=w_sb[:, 2 * k:2 * k + 2, hc * NH:(hc + 1) * NH],
                            start=(k == 0),
                            stop=(k == KP - 1),
                            perf_mode=mybir.MatmulPerfMode.DoubleRow,
                        )
                if e == 1 and mc == 0:
                    # Softmax transposes deferred here: the PE is 6.9us
                    # into e1-mc0 by now, so lg_sb is long ready.
                    emit_softmax()
                g_ap = gate_sb[:, mc * E + e:mc * E + e + 1]
                if e == 1:
                    # Deferred e0 scale: acc currently holds raw y0.
                    g0_ap = gate_sb[:, mc * E:mc * E + 1]
                    for hc in range(HC):
                        a_ap = acc_sb[:, mc * H + hc * NH:mc * H + (hc + 1) * NH]
                        nc.vector.tensor_scalar_mul(a_ap, a_ap, g0_ap)
                for hc in range(HC):
                    a_ap = acc_sb[:, mc * H + hc * NH:mc * H + (hc + 1) * NH]
                    if e < E - 1:
                        nc.vector.scalar_tensor_tensor(
                            a_ap, pss[hc][:], g_ap, a_ap,
                            op0=mybir.AluOpType.mult, op1=mybir.AluOpType.add,
                        )
                    else:
                        ot = outp.tile([128, NH], dt.bfloat16, tag="ot")
                        nc.vector.scalar_tensor_tensor(
                            ot[:], pss[hc][:], g_ap, a_ap,
                            op0=mybir.AluOpType.mult, op1=mybir.AluOpType.add,
                        )
                        out_q[mc][hc].dma_start(
                            out[msl, hc * NH:(hc + 1) * NH], ot[:]
                        )

    nc.compile()
    return nc


def _get_nc() -> bass.Bass:
    global _NC
    if _NC is None:
        _NC = _build_program()
    return _NC


def _prep_in_maps(x, gate_w, gate_b, expert_w):
    f8fn = ml_dtypes.float8_e4m3fn
    f8trn = ml_dtypes.float8_e4m3  # same bits as e4m3fn for |v| <= 240

    x = np.asarray(x, dtype=np.float32)
    gate_w = np.asarray(gate_w, dtype=np.float32)
    gate_b = np.asarray(gate_b, dtype=np.float32)
    expert_w = np.asarray(expert_w, dtype=np.float32)

    # x^T: [D, M]; quantized and fp16 (gating) copies.
    xT = np.ascontiguousarray(x.T)                       # [D, M] f32
    xT_f16 = xT.astype(np.float16)                       # [D, M] fp16 (gating)
    xqT = xT.astype(f8fn).view(f8trn)                    # [D, M] fp8
    # expert_w [E, H, D] -> w^T per expert [E, D, H], quantized, stacked.
    wqT = np.ascontiguousarray(
        expert_w.transpose(0, 2, 1)
    ).astype(f8fn).view(f8trn).reshape(E * D, H)
    gwt = np.ascontiguousarray(gate_w.T).astype(np.float16)  # [D, E] fp16
    gbb = np.ascontiguousarray(gate_b.reshape(E, 1))

    in_maps = []
    for c in range(NCORES):
        csl = slice(c * MS, (c + 1) * MS)
        in_maps.append({
            "xq": np.ascontiguousarray(xqT[:, csl]),
            "xf": np.ascontiguousarray(xT_f16[:, csl]),
            "wq": wqT,
            "gwt": gwt,
            "gb": gbb,
        })
    return in_maps


def kernel(x, gate_w, gate_b, expert_w, _trace=False, _trace_kwargs=None):
    nc = _get_nc()
    in_maps = _prep_in_maps(x, gate_w, gate_b, expert_w)
    kw = {}
    if _trace:
        kw["trace"] = True
        kw.update(_trace_kwargs or {})
    res = run_bass_kernel_spmd(nc, in_maps, core_ids=list(range(NCORES)), **kw)
    outp = np.concatenate(
        [np.asarray(res.results[c]["out"]).astype(np.float32)
         for c in range(NCORES)],
        axis=0,
    )
    if _trace:
        return outp, res
    return outp
